# revision 25
# baseline (speedup 1.0000x reference)
"""ANOVA kernel (order 3) on 8 TRN2 NeuronCores.

Math: out[b] = sum_e e3(x[b, :, e]) where e3 is the 3rd elementary
symmetric polynomial over the field axis. Via Newton's identities:
    e3 = (p1^3 - 3*p1*p2 + 2*p3) / 6,   p_k = sum_f x^k
so the kernel is: elementwise x^2 (ACT), x^3 (DVE), field-axis
reductions on TensorE (matmuls with one-hot selector weights, bf16),
then a small fused finale. Data parallel over batch: core c handles
b in [1024c, 1024c+1024).

Layout per core: tile tau covers 16 consecutive b. SBUF tile (128, 512):
partition p = b_q*32 + fp (b_q in [0,4), f-pair fp in [0,32)),
free n = j2*128 + parity*64 + e. This gives 512-byte contiguous DMA
descriptors (f-pairs): DRAM offset = p*128 + j2*16384 + parity*64 + e.

DMA: supers 0-1 load fp32 via SP HWDGE in the otherwise-dead window
before the Pool engine wakes (~5.9us); ACT makes their bf16 copies.
Supers 2-14 are single-call SWDGE cast-DMAs fp32->bf16 (full-super
calls so Pool descriptor generation stays ahead of the 16 DMA
queues); the last super is split tile-wise for a short drain tail.

TensorE: tile tau' of a fill accumulates into PSUM rows
64*(tau'%2) + 4*(tau'//2) + b_q. Consecutive tiles alternate PSUM
column-group halves (tile_position (0,0)/(0,64)), so back-to-back
same-stat matmuls run concurrently in disjoint column groups of the
PE array (2x effective matmul throughput). 32 tiles fill a (128, 512)
PSUM tensor per stat; one fill per half of the core's batch. The
finale adds the two f-parity halves, applies Newton's formula, and
reduces over e. The first fill's finale runs mid-kernel, hidden
under DMA.
"""

import sys

if "/opt/trn_rl_repo" not in sys.path:
    sys.path.insert(0, "/opt/trn_rl_repo")

import numpy as np

N_CORES = 8
B, F, E = 8192, 64, 64
B_PER_CORE = B // N_CORES  # 1024
J2 = 4                     # b-quads per tile
FD = 512                   # tile free dim = J2 * 2 * E (one PSUM bank)
TILES = B_PER_CORE // 16   # 64 (16 b per tile)
SUPER = 4                  # tiles per superblock for big ACT/DVE ops
N_SUPER = TILES // SUPER   # 16
SFD = FD * SUPER           # 2048
FILL_SUPERS = 8            # supers per PSUM fill (32 tiles = 128 rows)
HW_SUPERS = 2              # leading supers loaded as fp32 via SP HWDGE

_cache = {}


def _make_g() -> np.ndarray:
    """One-hot selector weights (128, 124) bf16: row k has a 1 at col
    60 + k//32. lhsT for tile tau' is g[:, 60-4*m16 : 124-4*m16] with
    m16 = tau' // 2, so lhsT[k, m] = 1 iff m == 4*m16 + k//32."""
    import ml_dtypes

    g = np.zeros((128, 124), dtype=ml_dtypes.bfloat16)
    for k in range(128):
        g[k, 60 + k // 32] = 1.0
    return g


def _build():
    import concourse.bass as bass
    import concourse.tile as tile
    from concourse import bacc, mybir

    nc = bacc.Bacc(
        "TRN2", target_bir_lowering=False, debug=False, num_devices=N_CORES
    )
    f32 = mybir.dt.float32
    bf16 = mybir.dt.bfloat16

    x_dram = nc.dram_tensor(
        "x", [B_PER_CORE, F, E], f32, kind="ExternalInput"
    ).ap()
    g_dram = nc.dram_tensor("g", [128, 124], bf16, kind="ExternalInput").ap()
    out_dram = nc.dram_tensor("out", [128, 2 * J2], f32, kind="ExternalOutput").ap()

    TILE_ELEMS = 16 * F * E  # 65536

    def x_ap(tile0: int, ntiles: int) -> bass.AP:
        ap = [[128, 128]]  # partition (b_q, fp): uniform stride 128
        if ntiles > 1:
            ap.append([TILE_ELEMS, ntiles])
        ap += [[4 * F * E, J2], [1, 2 * E]]  # j2, (parity e)
        return bass.AP(tensor=x_dram.tensor, offset=tile0 * TILE_ELEMS, ap=ap)

    with tile.TileContext(nc) as tc:
        with (
            tc.tile_pool(name="const", bufs=1) as const_pool,
            tc.tile_pool(name="xf32", bufs=2) as xf_pool,
            tc.tile_pool(name="xin", bufs=16) as x_pool,
            tc.tile_pool(name="xsq", bufs=6) as x2_pool,
            tc.tile_pool(name="xcu", bufs=6) as x3_pool,
            tc.tile_pool(name="acc", bufs=1, space="PSUM") as psum_pool,
            tc.tile_pool(name="tail", bufs=2) as tail_pool,
        ):
            g_sb = const_pool.tile([128, 124], bf16)
            outt = const_pool.tile([128, 2 * J2], f32)

            # fp32 staging for two HWDGE-preloaded leading supers: HWDGE
            # fp32 descriptors cost ~43ns vs ~23 for SWDGE cast, but they
            # run in the otherwise-dead window before the Pool engine
            # wakes (~5.9us) and prime the queues; measured net positive.
            xf32 = [
                xf_pool.tile([128, SFD], f32, name="xf32")
                for _ in range(HW_SUPERS)
            ]
            nc.sync.dma_start(out=xf32[0][:], in_=x_ap(0, SUPER))
            nc.sync.dma_start(out=xf32[1][:], in_=x_ap(SUPER, SUPER))
            nc.sync.dma_start(out=g_sb[:], in_=g_dram[:])

            psums = [
                [
                    psum_pool.tile([128, FD], f32, name=f"psum_{phi}_{stat}")
                    for stat in range(3)
                ]
                for phi in range(2)
            ]

            def finale(phi: int):
                """e3 = (p1^3 - 3 p1 p2 + 2 p3)/6 summed over e, for one
                PSUM fill. Starts by summing the two f-parity halves."""
                p1t, p2t, p3t = psums[phi]
                pa = []
                for idx, pt in enumerate((p1t, p2t)):
                    v = pt[:].rearrange("p (j t e) -> p j t e", j=J2, t=2)
                    a = tail_pool.tile([128, J2, E], f32, name=f"pa{idx}")
                    # DVE can read only one PSUM operand: stage parity 0
                    # through ACT, then add parity 1 (PSUM) on DVE.
                    nc.scalar.copy(a[:], v[:, :, 0, :])
                    nc.vector.tensor_add(a[:], a[:], v[:, :, 1, :])
                    pa.append(a)
                pa1, pa2 = pa
                # sum_e(u3 + 2*p3) = sum_e(u3) + 2*sum_e(p3): reduce p3
                # early (off the critical chain) and combine at the end.
                # The final /6 happens on the host (bit-exact fp32 mul).
                v3 = p3t[:].rearrange("p (j t e) -> p j t e", j=J2, t=2)
                r3p = tail_pool.tile([128, J2, 2], f32)
                nc.vector.reduce_sum(r3p[:], v3, axis=mybir.AxisListType.X)
                red3 = tail_pool.tile([128, J2], f32)
                nc.vector.tensor_add(red3[:], r3p[:, :, 0], r3p[:, :, 1])
                t1 = tail_pool.tile([128, J2 * E], f32)
                nc.scalar.square(t1[:], pa1[:])  # p1^2
                u2 = tail_pool.tile([128, J2 * E], f32)
                nc.vector.scalar_tensor_tensor(  # p1^2 - 3 p2
                    u2[:], pa2[:], -3.0, t1[:],
                    op0=mybir.AluOpType.mult, op1=mybir.AluOpType.add,
                )
                u3 = tail_pool.tile([128, J2 * E], f32)
                nc.vector.tensor_mul(u3[:], u2[:], pa1[:])  # p1^3 - 3 p1 p2
                redu = tail_pool.tile([128, J2], f32)
                nc.vector.reduce_sum(
                    redu[:],
                    u3[:].rearrange("p (j e) -> p j e", j=J2),
                    axis=mybir.AxisListType.X,
                )
                nc.vector.scalar_tensor_tensor(  # 2*sum(p3) + sum(u3)
                    outt[:, J2 * phi : J2 * (phi + 1)], red3[:], 2.0, redu[:],
                    op0=mybir.AluOpType.mult, op1=mybir.AluOpType.add,
                )
                # per-fill output store: fill 0's half is hidden mid-kernel
                nc.sync.dma_start(
                    out=out_dram[:, J2 * phi : J2 * (phi + 1)],
                    in_=outt[:, J2 * phi : J2 * (phi + 1)],
                )

            def mm(phi: int, stat: int, taup: int, src, k: int):
                """One selector matmul: tile tau' of fill phi, stat s.
                PSUM rows 64*(taup%2) + 4*(taup//2) + b_q; consecutive
                taup alternate column-group halves for PE concurrency."""
                pm, m16 = taup % 2, taup // 2
                nc.tensor.matmul(
                    psums[phi][stat][64 * pm : 64 * pm + 64, :],
                    g_sb[:, 60 - 4 * m16 : 124 - 4 * m16],
                    src[:, k * FD : (k + 1) * FD],
                    start=m16 == 0,
                    stop=m16 == 15,
                    tile_position=(0, 64 * pm),
                    skip_group_check=True,
                )

            # squares computed on DVE (not ACT) for these supers: ACT
            # (~0.96 GHz, 1 elem/cycle) is otherwise the busiest engine
            # super 14 on DVE so ACT is free when the split last
            # super's tiles arrive (tail squares start at chunk arrival)
            DVE_SQUARE_SUPERS = {2, 5, 8, 11, 14}

            def compute(s: int, xb, chunks: int):
                """Square/cube + matmuls for super s. xb is the bf16
                x tile. chunks>1 splits ops tile-wise (drain tail)."""
                phi = s // FILL_SUPERS
                x2b = x2_pool.tile([128, SFD], bf16, name="x2b")
                x3b = x3_pool.tile([128, SFD], bf16, name="x3b")
                csz = SFD // chunks
                for c in range(chunks):
                    cs = slice(c * csz, (c + 1) * csz)
                    if s in DVE_SQUARE_SUPERS:
                        nc.vector.tensor_mul(x2b[:, cs], xb[:, cs], xb[:, cs])
                    else:
                        nc.scalar.square(x2b[:, cs], xb[:, cs])
                    nc.vector.tensor_mul(x3b[:, cs], x2b[:, cs], xb[:, cs])
                    if chunks > 1:  # tail mode: per-chunk matmuls ASAP
                        tpc = SUPER // chunks
                        for k in range(c * tpc, (c + 1) * tpc):
                            taup = (s % FILL_SUPERS) * SUPER + k
                            for stat, srt in ((0, xb), (1, x2b), (2, x3b)):
                                mm(phi, stat, taup, srt, k)
                if chunks == 1:
                    # stat-major order: consecutive k alternate column
                    # groups -> pairs run concurrently on the PE
                    for stat, src in ((0, xb), (1, x2b), (2, x3b)):
                        for k in range(SUPER):
                            taup = (s % FILL_SUPERS) * SUPER + k
                            mm(phi, stat, taup, src, k)
                if s % FILL_SUPERS == FILL_SUPERS - 1:
                    finale(s // FILL_SUPERS)

            for s in range(N_SUPER):
                xb = x_pool.tile([128, SFD], bf16, name="xb")
                if s < HW_SUPERS:
                    # bf16 copy of the fp32-staged super on ACT
                    half = SFD // 2
                    for c in range(2):
                        cs = slice(c * half, (c + 1) * half)
                        nc.scalar.copy(xb[:, cs], xf32[s][:, cs])
                    compute(s, xb, chunks=1)
                elif s == N_SUPER - 1:
                    # split the last super tile-wise for a short tail
                    for c in range(SUPER):
                        nc.gpsimd.dma_start(
                            out=xb[:, c * FD : (c + 1) * FD],
                            in_=x_ap(s * SUPER + c, 1),
                        )
                    compute(s, xb, chunks=SUPER)
                else:
                    nc.gpsimd.dma_start(out=xb[:], in_=x_ap(s * SUPER, SUPER))
                    compute(s, xb, chunks=1)

    nc.compile()
    return nc


def _get_nc():
    if "nc" not in _cache:
        _cache["nc"] = _build()
    return _cache["nc"]


def _unpermute(r: np.ndarray) -> np.ndarray:
    # r[row, J2*phi + j2] is the value for b = 512*phi + 16*tau' +
    # 4*j2 + b_q with row = 64*(tau'%2) + 4*(tau'//2) + b_q.
    row = np.arange(128)
    tau = 2 * ((row % 64) // 4) + row // 64
    b_q = row % 4
    out = np.empty(B_PER_CORE, dtype=r.dtype)
    for phi in range(2):
        for j2 in range(J2):
            out[512 * phi + 16 * tau + 4 * j2 + b_q] = r[:, J2 * phi + j2]
    return out


def _run(x: np.ndarray, **kwargs):
    from concourse.bass_utils import run_bass_kernel_spmd

    nc = _get_nc()
    g = _make_g()
    shards = x.reshape(N_CORES, B_PER_CORE, F, E)
    in_maps = [
        {"x": np.ascontiguousarray(shards[c]), "g": g} for c in range(N_CORES)
    ]
    res = run_bass_kernel_spmd(nc, in_maps, core_ids=list(range(N_CORES)), **kwargs)
    out = np.concatenate(
        [_unpermute(np.asarray(res.results[c]["out"])) for c in range(N_CORES)]
    ).astype(np.float32)
    out *= np.float32(1.0 / 6.0)  # the /6 of Newton's formula, host-side
    return out, res


def kernel(**inputs) -> np.ndarray:
    x = np.ascontiguousarray(np.asarray(inputs["x"], dtype=np.float32))
    assert x.shape == (B, F, E), x.shape
    out, _ = _run(x)
    return out


# revision 26
# speedup vs baseline: 1.0319x; 1.0319x over previous
"""ANOVA kernel (order 3) on 8 TRN2 NeuronCores.

Math: out[b] = sum_e e3(x[b, :, e]) where e3 is the 3rd elementary
symmetric polynomial over the field axis. Via Newton's identities:
    e3 = (p1^3 - 3*p1*p2 + 2*p3) / 6,   p_k = sum_f x^k
so the kernel is: elementwise x^2 (ACT), x^3 (DVE), field-axis
reductions on TensorE (matmuls with one-hot selector weights, bf16),
then a small fused finale. Data parallel over batch: core c handles
b in [1024c, 1024c+1024).

Layout per core: tile tau covers 16 consecutive b. SBUF tile (128, 512):
partition p = b_q*32 + fp (b_q in [0,4), f-pair fp in [0,32)),
free n = j2*128 + parity*64 + e. This gives 512-byte contiguous DMA
descriptors (f-pairs): DRAM offset = p*128 + j2*16384 + parity*64 + e.

DMA: supers 0-1 load fp32 via SP HWDGE in the otherwise-dead window
before the Pool engine wakes (~5.9us); ACT makes their bf16 copies.
Supers 2-14 are single-call SWDGE cast-DMAs fp32->bf16 (full-super
calls so Pool descriptor generation stays ahead of the 16 DMA
queues); the last super is split tile-wise for a short drain tail.

TensorE: tile tau' of a fill accumulates into PSUM rows
64*(tau'%2) + 4*(tau'//2) + b_q. Consecutive tiles alternate PSUM
column-group halves (tile_position (0,0)/(0,64)), so back-to-back
same-stat matmuls run concurrently in disjoint column groups of the
PE array (2x effective matmul throughput). 32 tiles fill a (128, 512)
PSUM tensor per stat; one fill per half of the core's batch. The
finale adds the two f-parity halves, applies Newton's formula, and
reduces over e. The first fill's finale runs mid-kernel, hidden
under DMA.
"""

import sys

if "/opt/trn_rl_repo" not in sys.path:
    sys.path.insert(0, "/opt/trn_rl_repo")

import numpy as np

N_CORES = 8
B, F, E = 8192, 64, 64
B_PER_CORE = B // N_CORES  # 1024
J2 = 4                     # b-quads per tile
FD = 512                   # tile free dim = J2 * 2 * E (one PSUM bank)
TILES = B_PER_CORE // 16   # 64 (16 b per tile)
SUPER = 4                  # tiles per superblock for big ACT/DVE ops
N_SUPER = TILES // SUPER   # 16
SFD = FD * SUPER           # 2048
FILL_SUPERS = 8            # supers per PSUM fill (32 tiles = 128 rows)
HW_SUPERS = 2              # leading supers loaded as fp32 via SP HWDGE

_cache = {}


def _make_g() -> np.ndarray:
    """One-hot selector weights (128, 124) bf16: row k has a 1 at col
    60 + k//32. lhsT for tile tau' is g[:, 60-4*m16 : 124-4*m16] with
    m16 = tau' // 2, so lhsT[k, m] = 1 iff m == 4*m16 + k//32."""
    import ml_dtypes

    g = np.zeros((128, 124), dtype=ml_dtypes.bfloat16)
    for k in range(128):
        g[k, 60 + k // 32] = 1.0
    return g


def _build():
    import concourse.bass as bass
    import concourse.tile as tile
    from concourse import bacc, mybir

    nc = bacc.Bacc(
        "TRN2", target_bir_lowering=False, debug=False, num_devices=N_CORES
    )
    f32 = mybir.dt.float32
    bf16 = mybir.dt.bfloat16

    x_dram = nc.dram_tensor(
        "x", [B_PER_CORE, F, E], f32, kind="ExternalInput"
    ).ap()
    g_dram = nc.dram_tensor("g", [128, 124], bf16, kind="ExternalInput").ap()
    out_dram = nc.dram_tensor("out", [128, 2 * J2], f32, kind="ExternalOutput").ap()

    TILE_ELEMS = 16 * F * E  # 65536

    def x_ap(tile0: int, ntiles: int) -> bass.AP:
        ap = [[128, 128]]  # partition (b_q, fp): uniform stride 128
        if ntiles > 1:
            ap.append([TILE_ELEMS, ntiles])
        ap += [[4 * F * E, J2], [1, 2 * E]]  # j2, (parity e)
        return bass.AP(tensor=x_dram.tensor, offset=tile0 * TILE_ELEMS, ap=ap)

    with tile.TileContext(nc) as tc:
        with (
            tc.tile_pool(name="const", bufs=1) as const_pool,
            tc.tile_pool(name="xf32", bufs=2) as xf_pool,
            tc.tile_pool(name="xin", bufs=16) as x_pool,
            tc.tile_pool(name="xsq", bufs=6) as x2_pool,
            tc.tile_pool(name="xcu", bufs=6) as x3_pool,
            tc.tile_pool(name="acc", bufs=1, space="PSUM") as psum_pool,
            tc.tile_pool(name="tail", bufs=2) as tail_pool,
        ):
            g_sb = const_pool.tile([128, 124], bf16)
            outt = const_pool.tile([128, 2 * J2], f32)

            # fp32 staging for two HWDGE-preloaded leading supers: HWDGE
            # fp32 descriptors cost ~43ns vs ~23 for SWDGE cast, but they
            # run in the otherwise-dead window before the Pool engine
            # wakes (~5.9us) and prime the queues; measured net positive.
            xf32 = [
                xf_pool.tile([128, SFD], f32, name="xf32")
                for _ in range(HW_SUPERS)
            ]
            nc.sync.dma_start(out=xf32[0][:], in_=x_ap(0, SUPER))
            nc.sync.dma_start(out=xf32[1][:], in_=x_ap(SUPER, SUPER))
            nc.sync.dma_start(out=g_sb[:], in_=g_dram[:])

            psums = [
                [
                    psum_pool.tile([128, FD], f32, name=f"psum_{phi}_{stat}")
                    for stat in range(3)
                ]
                for phi in range(2)
            ]

            def finale(phi: int):
                """e3 = (p1^3 - 3 p1 p2 + 2 p3)/6 summed over e, for one
                PSUM fill. Starts by summing the two f-parity halves."""
                p1t, p2t, p3t = psums[phi]
                pa = []
                for idx, pt in enumerate((p1t, p2t)):
                    v = pt[:].rearrange("p (j t e) -> p j t e", j=J2, t=2)
                    a = tail_pool.tile([128, J2, E], f32, name=f"pa{idx}")
                    # DVE can read only one PSUM operand: stage parity 0
                    # through ACT, then add parity 1 (PSUM) on DVE.
                    nc.scalar.copy(a[:], v[:, :, 0, :])
                    nc.vector.tensor_add(a[:], a[:], v[:, :, 1, :])
                    pa.append(a)
                pa1, pa2 = pa
                # sum_e(u3 + 2*p3) = sum_e(u3) + 2*sum_e(p3): reduce p3
                # early (off the critical chain) and combine at the end.
                # The final /6 happens on the host (bit-exact fp32 mul).
                v3 = p3t[:].rearrange("p (j t e) -> p j t e", j=J2, t=2)
                r3p = tail_pool.tile([128, J2, 2], f32)
                nc.vector.reduce_sum(r3p[:], v3, axis=mybir.AxisListType.X)
                red3 = tail_pool.tile([128, J2], f32)
                nc.vector.tensor_add(red3[:], r3p[:, :, 0], r3p[:, :, 1])
                t1 = tail_pool.tile([128, J2 * E], f32)
                nc.scalar.square(t1[:], pa1[:])  # p1^2
                u2 = tail_pool.tile([128, J2 * E], f32)
                nc.vector.scalar_tensor_tensor(  # p1^2 - 3 p2
                    u2[:], pa2[:], -3.0, t1[:],
                    op0=mybir.AluOpType.mult, op1=mybir.AluOpType.add,
                )
                u3 = tail_pool.tile([128, J2 * E], f32)
                nc.vector.tensor_mul(u3[:], u2[:], pa1[:])  # p1^3 - 3 p1 p2
                redu = tail_pool.tile([128, J2], f32)
                nc.vector.reduce_sum(
                    redu[:],
                    u3[:].rearrange("p (j e) -> p j e", j=J2),
                    axis=mybir.AxisListType.X,
                )
                nc.vector.scalar_tensor_tensor(  # 2*sum(p3) + sum(u3)
                    outt[:, J2 * phi : J2 * (phi + 1)], red3[:], 2.0, redu[:],
                    op0=mybir.AluOpType.mult, op1=mybir.AluOpType.add,
                )
                # per-fill output store: fill 0's half is hidden mid-kernel
                nc.sync.dma_start(
                    out=out_dram[:, J2 * phi : J2 * (phi + 1)],
                    in_=outt[:, J2 * phi : J2 * (phi + 1)],
                )

            def mm(phi: int, stat: int, taup: int, src, k: int):
                """One selector matmul: tile tau' of fill phi, stat s.
                PSUM rows 64*(taup%2) + 4*(taup//2) + b_q; consecutive
                taup alternate column-group halves for PE concurrency."""
                pm, m16 = taup % 2, taup // 2
                nc.tensor.matmul(
                    psums[phi][stat][64 * pm : 64 * pm + 64, :],
                    g_sb[:, 60 - 4 * m16 : 124 - 4 * m16],
                    src[:, k * FD : (k + 1) * FD],
                    start=m16 == 0,
                    stop=m16 == 15,
                    tile_position=(0, 64 * pm),
                    skip_group_check=True,
                )

            # squares computed on DVE (not ACT) for these supers: ACT
            # (~0.96 GHz, 1 elem/cycle) is otherwise the busiest engine
            # squares on DVE only mid-stream: the drain tail is
            # DVE-bound (cube chain + finale), so supers 12-15 must
            # stay on ACT; measured worse with any of them on DVE
            DVE_SQUARE_SUPERS = {2, 5, 8, 11}

            def compute(s: int, xb, chunks: int):
                """Square/cube + matmuls for super s. xb is the bf16
                x tile. chunks>1 splits ops tile-wise (drain tail)."""
                phi = s // FILL_SUPERS
                x2b = x2_pool.tile([128, SFD], bf16, name="x2b")
                x3b = x3_pool.tile([128, SFD], bf16, name="x3b")
                csz = SFD // chunks
                for c in range(chunks):
                    cs = slice(c * csz, (c + 1) * csz)
                    if s in DVE_SQUARE_SUPERS:
                        nc.vector.tensor_mul(x2b[:, cs], xb[:, cs], xb[:, cs])
                    else:
                        nc.scalar.square(x2b[:, cs], xb[:, cs])
                    nc.vector.tensor_mul(x3b[:, cs], x2b[:, cs], xb[:, cs])
                    if chunks > 1:  # tail mode: per-chunk matmuls ASAP
                        tpc = SUPER // chunks
                        for k in range(c * tpc, (c + 1) * tpc):
                            taup = (s % FILL_SUPERS) * SUPER + k
                            for stat, srt in ((0, xb), (1, x2b), (2, x3b)):
                                mm(phi, stat, taup, srt, k)
                if chunks == 1:
                    # stat-major order: consecutive k alternate column
                    # groups -> pairs run concurrently on the PE
                    for stat, src in ((0, xb), (1, x2b), (2, x3b)):
                        for k in range(SUPER):
                            taup = (s % FILL_SUPERS) * SUPER + k
                            mm(phi, stat, taup, src, k)
                if s % FILL_SUPERS == FILL_SUPERS - 1:
                    finale(s // FILL_SUPERS)

            for s in range(N_SUPER):
                xb = x_pool.tile([128, SFD], bf16, name="xb")
                if s < HW_SUPERS:
                    # bf16 copy of the fp32-staged super on ACT
                    half = SFD // 2
                    for c in range(2):
                        cs = slice(c * half, (c + 1) * half)
                        nc.scalar.copy(xb[:, cs], xf32[s][:, cs])
                    compute(s, xb, chunks=1)
                elif s == N_SUPER - 1:
                    # split the last super tile-wise for a short tail
                    for c in range(SUPER):
                        nc.gpsimd.dma_start(
                            out=xb[:, c * FD : (c + 1) * FD],
                            in_=x_ap(s * SUPER + c, 1),
                        )
                    compute(s, xb, chunks=SUPER)
                else:
                    nc.gpsimd.dma_start(out=xb[:], in_=x_ap(s * SUPER, SUPER))
                    compute(s, xb, chunks=1)

    nc.compile()
    return nc


def _get_nc():
    if "nc" not in _cache:
        _cache["nc"] = _build()
    return _cache["nc"]


def _unpermute(r: np.ndarray) -> np.ndarray:
    # r[row, J2*phi + j2] is the value for b = 512*phi + 16*tau' +
    # 4*j2 + b_q with row = 64*(tau'%2) + 4*(tau'//2) + b_q.
    row = np.arange(128)
    tau = 2 * ((row % 64) // 4) + row // 64
    b_q = row % 4
    out = np.empty(B_PER_CORE, dtype=r.dtype)
    for phi in range(2):
        for j2 in range(J2):
            out[512 * phi + 16 * tau + 4 * j2 + b_q] = r[:, J2 * phi + j2]
    return out


def _run(x: np.ndarray, **kwargs):
    from concourse.bass_utils import run_bass_kernel_spmd

    nc = _get_nc()
    g = _make_g()
    shards = x.reshape(N_CORES, B_PER_CORE, F, E)
    in_maps = [
        {"x": np.ascontiguousarray(shards[c]), "g": g} for c in range(N_CORES)
    ]
    res = run_bass_kernel_spmd(nc, in_maps, core_ids=list(range(N_CORES)), **kwargs)
    out = np.concatenate(
        [_unpermute(np.asarray(res.results[c]["out"])) for c in range(N_CORES)]
    ).astype(np.float32)
    out *= np.float32(1.0 / 6.0)  # the /6 of Newton's formula, host-side
    return out, res


def kernel(**inputs) -> np.ndarray:
    x = np.ascontiguousarray(np.asarray(inputs["x"], dtype=np.float32))
    assert x.shape == (B, F, E), x.shape
    out, _ = _run(x)
    return out


# revision 27
# speedup vs baseline: 1.0450x; 1.0127x over previous
"""ANOVA kernel (order 3) on 8 TRN2 NeuronCores.

Math: out[b] = sum_e e3(x[b, :, e]) where e3 is the 3rd elementary
symmetric polynomial over the field axis. Via Newton's identities:
    e3 = (p1^3 - 3*p1*p2 + 2*p3) / 6,   p_k = sum_f x^k
so the kernel is: elementwise x^2 (ACT), x^3 (DVE), field-axis
reductions on TensorE (matmuls with one-hot selector weights, bf16),
then a small fused finale. Data parallel over batch: core c handles
b in [1024c, 1024c+1024).

Layout per core: tile tau covers 16 consecutive b. SBUF tile (128, 512):
partition p = b_q*32 + fp (b_q in [0,4), f-pair fp in [0,32)),
free n = j2*128 + parity*64 + e. This gives 512-byte contiguous DMA
descriptors (f-pairs): DRAM offset = p*128 + j2*16384 + parity*64 + e.

DMA: supers 0-1 load fp32 via SP HWDGE in the otherwise-dead window
before the Pool engine wakes (~5.9us); ACT makes their bf16 copies.
Supers 2-14 are single-call SWDGE cast-DMAs fp32->bf16 (full-super
calls so Pool descriptor generation stays ahead of the 16 DMA
queues); the last super is split tile-wise for a short drain tail.

TensorE: tile tau' of a fill accumulates into PSUM rows
64*(tau'%2) + 4*(tau'//2) + b_q. Consecutive tiles alternate PSUM
column-group halves (tile_position (0,0)/(0,64)), so back-to-back
same-stat matmuls run concurrently in disjoint column groups of the
PE array (2x effective matmul throughput). 32 tiles fill a (128, 512)
PSUM tensor per stat; one fill per half of the core's batch. The
finale adds the two f-parity halves, applies Newton's formula, and
reduces over e. The first fill's finale runs mid-kernel, hidden
under DMA.
"""

import sys

if "/opt/trn_rl_repo" not in sys.path:
    sys.path.insert(0, "/opt/trn_rl_repo")

import numpy as np

N_CORES = 8
B, F, E = 8192, 64, 64
B_PER_CORE = B // N_CORES  # 1024
J2 = 4                     # b-quads per tile
FD = 512                   # tile free dim = J2 * 2 * E (one PSUM bank)
TILES = B_PER_CORE // 16   # 64 (16 b per tile)
SUPER = 4                  # tiles per superblock for big ACT/DVE ops
N_SUPER = TILES // SUPER   # 16
SFD = FD * SUPER           # 2048
FILL_SUPERS = 8            # supers per PSUM fill (32 tiles = 128 rows)
HW_SUPERS = 1              # leading supers loaded as fp32 via SP HWDGE

_cache = {}


def _make_g() -> np.ndarray:
    """One-hot selector weights (128, 124) bf16: row k has a 1 at col
    60 + k//32. lhsT for tile tau' is g[:, 60-4*m16 : 124-4*m16] with
    m16 = tau' // 2, so lhsT[k, m] = 1 iff m == 4*m16 + k//32."""
    import ml_dtypes

    g = np.zeros((128, 124), dtype=ml_dtypes.bfloat16)
    for k in range(128):
        g[k, 60 + k // 32] = 1.0
    return g


def _build():
    import concourse.bass as bass
    import concourse.tile as tile
    from concourse import bacc, mybir

    nc = bacc.Bacc(
        "TRN2", target_bir_lowering=False, debug=False, num_devices=N_CORES
    )
    f32 = mybir.dt.float32
    bf16 = mybir.dt.bfloat16

    x_dram = nc.dram_tensor(
        "x", [B_PER_CORE, F, E], f32, kind="ExternalInput"
    ).ap()
    g_dram = nc.dram_tensor("g", [128, 124], bf16, kind="ExternalInput").ap()
    out_dram = nc.dram_tensor("out", [128, 2 * J2], f32, kind="ExternalOutput").ap()

    TILE_ELEMS = 16 * F * E  # 65536

    def x_ap(tile0: int, ntiles: int) -> bass.AP:
        ap = [[128, 128]]  # partition (b_q, fp): uniform stride 128
        if ntiles > 1:
            ap.append([TILE_ELEMS, ntiles])
        ap += [[4 * F * E, J2], [1, 2 * E]]  # j2, (parity e)
        return bass.AP(tensor=x_dram.tensor, offset=tile0 * TILE_ELEMS, ap=ap)

    with tile.TileContext(nc) as tc:
        with (
            tc.tile_pool(name="const", bufs=1) as const_pool,
            tc.tile_pool(name="xf32", bufs=2) as xf_pool,
            tc.tile_pool(name="xin", bufs=16) as x_pool,
            tc.tile_pool(name="xsq", bufs=6) as x2_pool,
            tc.tile_pool(name="xcu", bufs=6) as x3_pool,
            tc.tile_pool(name="acc", bufs=1, space="PSUM") as psum_pool,
            tc.tile_pool(name="tail", bufs=2) as tail_pool,
        ):
            g_sb = const_pool.tile([128, 124], bf16)
            outt = const_pool.tile([128, 2 * J2], f32)

            # fp32 staging for two HWDGE-preloaded leading supers: HWDGE
            # fp32 descriptors cost ~43ns vs ~23 for SWDGE cast, but they
            # run in the otherwise-dead window before the Pool engine
            # wakes (~5.9us) and prime the queues; measured net positive.
            xf32 = [
                xf_pool.tile([128, SFD], f32, name="xf32")
                for _ in range(HW_SUPERS)
            ]
            for s in range(HW_SUPERS):
                nc.sync.dma_start(out=xf32[s][:], in_=x_ap(s * SUPER, SUPER))
            nc.sync.dma_start(out=g_sb[:], in_=g_dram[:])

            psums = [
                [
                    psum_pool.tile([128, FD], f32, name=f"psum_{phi}_{stat}")
                    for stat in range(3)
                ]
                for phi in range(2)
            ]

            def finale(phi: int):
                """e3 = (p1^3 - 3 p1 p2 + 2 p3)/6 summed over e, for one
                PSUM fill. Starts by summing the two f-parity halves."""
                p1t, p2t, p3t = psums[phi]
                pa = []
                for idx, pt in enumerate((p1t, p2t)):
                    v = pt[:].rearrange("p (j t e) -> p j t e", j=J2, t=2)
                    a = tail_pool.tile([128, J2, E], f32, name=f"pa{idx}")
                    # DVE can read only one PSUM operand: stage parity 0
                    # through ACT, then add parity 1 (PSUM) on DVE.
                    nc.scalar.copy(a[:], v[:, :, 0, :])
                    nc.vector.tensor_add(a[:], a[:], v[:, :, 1, :])
                    pa.append(a)
                pa1, pa2 = pa
                # sum_e(u3 + 2*p3) = sum_e(u3) + 2*sum_e(p3): reduce p3
                # early (off the critical chain) and combine at the end.
                # The final /6 happens on the host (bit-exact fp32 mul).
                v3 = p3t[:].rearrange("p (j t e) -> p j t e", j=J2, t=2)
                r3p = tail_pool.tile([128, J2, 2], f32)
                nc.vector.reduce_sum(r3p[:], v3, axis=mybir.AxisListType.X)
                red3 = tail_pool.tile([128, J2], f32)
                nc.vector.tensor_add(red3[:], r3p[:, :, 0], r3p[:, :, 1])
                t1 = tail_pool.tile([128, J2 * E], f32)
                nc.scalar.square(t1[:], pa1[:])  # p1^2
                u2 = tail_pool.tile([128, J2 * E], f32)
                nc.vector.scalar_tensor_tensor(  # p1^2 - 3 p2
                    u2[:], pa2[:], -3.0, t1[:],
                    op0=mybir.AluOpType.mult, op1=mybir.AluOpType.add,
                )
                u3 = tail_pool.tile([128, J2 * E], f32)
                nc.vector.tensor_mul(u3[:], u2[:], pa1[:])  # p1^3 - 3 p1 p2
                redu = tail_pool.tile([128, J2], f32)
                nc.vector.reduce_sum(
                    redu[:],
                    u3[:].rearrange("p (j e) -> p j e", j=J2),
                    axis=mybir.AxisListType.X,
                )
                nc.vector.scalar_tensor_tensor(  # 2*sum(p3) + sum(u3)
                    outt[:, J2 * phi : J2 * (phi + 1)], red3[:], 2.0, redu[:],
                    op0=mybir.AluOpType.mult, op1=mybir.AluOpType.add,
                )
                # per-fill output store: fill 0's half is hidden mid-kernel
                nc.sync.dma_start(
                    out=out_dram[:, J2 * phi : J2 * (phi + 1)],
                    in_=outt[:, J2 * phi : J2 * (phi + 1)],
                )

            def mm(phi: int, stat: int, taup: int, src, k: int):
                """One selector matmul: tile tau' of fill phi, stat s.
                PSUM rows 64*(taup%2) + 4*(taup//2) + b_q; consecutive
                taup alternate column-group halves for PE concurrency."""
                pm, m16 = taup % 2, taup // 2
                nc.tensor.matmul(
                    psums[phi][stat][64 * pm : 64 * pm + 64, :],
                    g_sb[:, 60 - 4 * m16 : 124 - 4 * m16],
                    src[:, k * FD : (k + 1) * FD],
                    start=m16 == 0,
                    stop=m16 == 15,
                    tile_position=(0, 64 * pm),
                    skip_group_check=True,
                )

            # squares computed on DVE (not ACT) for these supers: ACT
            # (~0.96 GHz, 1 elem/cycle) is otherwise the busiest engine
            # squares on DVE only mid-stream: the drain tail is
            # DVE-bound (cube chain + finale), so supers 12-15 must
            # stay on ACT; measured worse with any of them on DVE
            DVE_SQUARE_SUPERS = {2, 5, 8, 11}

            def compute(s: int, xb, chunks: int):
                """Square/cube + matmuls for super s. xb is the bf16
                x tile. chunks>1 splits ops tile-wise (drain tail)."""
                phi = s // FILL_SUPERS
                x2b = x2_pool.tile([128, SFD], bf16, name="x2b")
                x3b = x3_pool.tile([128, SFD], bf16, name="x3b")
                csz = SFD // chunks
                for c in range(chunks):
                    cs = slice(c * csz, (c + 1) * csz)
                    if s in DVE_SQUARE_SUPERS:
                        nc.vector.tensor_mul(x2b[:, cs], xb[:, cs], xb[:, cs])
                    else:
                        nc.scalar.square(x2b[:, cs], xb[:, cs])
                    nc.vector.tensor_mul(x3b[:, cs], x2b[:, cs], xb[:, cs])
                    if chunks > 1:  # tail mode: per-chunk matmuls ASAP
                        tpc = SUPER // chunks
                        for k in range(c * tpc, (c + 1) * tpc):
                            taup = (s % FILL_SUPERS) * SUPER + k
                            for stat, srt in ((0, xb), (1, x2b), (2, x3b)):
                                mm(phi, stat, taup, srt, k)
                if chunks == 1:
                    # stat-major order: consecutive k alternate column
                    # groups -> pairs run concurrently on the PE
                    for stat, src in ((0, xb), (1, x2b), (2, x3b)):
                        for k in range(SUPER):
                            taup = (s % FILL_SUPERS) * SUPER + k
                            mm(phi, stat, taup, src, k)
                if s % FILL_SUPERS == FILL_SUPERS - 1:
                    finale(s // FILL_SUPERS)

            for s in range(N_SUPER):
                xb = x_pool.tile([128, SFD], bf16, name="xb")
                if s < HW_SUPERS:
                    # bf16 copy of the fp32-staged super on ACT
                    half = SFD // 2
                    for c in range(2):
                        cs = slice(c * half, (c + 1) * half)
                        nc.scalar.copy(xb[:, cs], xf32[s][:, cs])
                    compute(s, xb, chunks=1)
                elif s == N_SUPER - 1:
                    # split the last super tile-wise for a short tail
                    for c in range(SUPER):
                        nc.gpsimd.dma_start(
                            out=xb[:, c * FD : (c + 1) * FD],
                            in_=x_ap(s * SUPER + c, 1),
                        )
                    compute(s, xb, chunks=SUPER)
                else:
                    nc.gpsimd.dma_start(out=xb[:], in_=x_ap(s * SUPER, SUPER))
                    compute(s, xb, chunks=1)

    nc.compile()
    return nc


def _get_nc():
    if "nc" not in _cache:
        _cache["nc"] = _build()
    return _cache["nc"]


def _unpermute(r: np.ndarray) -> np.ndarray:
    # r[row, J2*phi + j2] is the value for b = 512*phi + 16*tau' +
    # 4*j2 + b_q with row = 64*(tau'%2) + 4*(tau'//2) + b_q.
    row = np.arange(128)
    tau = 2 * ((row % 64) // 4) + row // 64
    b_q = row % 4
    out = np.empty(B_PER_CORE, dtype=r.dtype)
    for phi in range(2):
        for j2 in range(J2):
            out[512 * phi + 16 * tau + 4 * j2 + b_q] = r[:, J2 * phi + j2]
    return out


def _run(x: np.ndarray, **kwargs):
    from concourse.bass_utils import run_bass_kernel_spmd

    nc = _get_nc()
    g = _make_g()
    shards = x.reshape(N_CORES, B_PER_CORE, F, E)
    in_maps = [
        {"x": np.ascontiguousarray(shards[c]), "g": g} for c in range(N_CORES)
    ]
    res = run_bass_kernel_spmd(nc, in_maps, core_ids=list(range(N_CORES)), **kwargs)
    out = np.concatenate(
        [_unpermute(np.asarray(res.results[c]["out"])) for c in range(N_CORES)]
    ).astype(np.float32)
    out *= np.float32(1.0 / 6.0)  # the /6 of Newton's formula, host-side
    return out, res


def kernel(**inputs) -> np.ndarray:
    x = np.ascontiguousarray(np.asarray(inputs["x"], dtype=np.float32))
    assert x.shape == (B, F, E), x.shape
    out, _ = _run(x)
    return out
